# revision 1
# baseline (speedup 1.0000x reference)
"""Dense MLP kernel for Trainium2: y = inputs @ kernel + bias.

Full shapes: inputs (4, 2048, 4096) f32, kernel (4096, 16384) f32,
bias (16384,) f32 -> y (4, 2048, 16384) f32.

Strategy: tensor-parallel over the output feature dim F=16384, split 8
ways (2048 features per core). Each core receives the full activations
(pre-transposed on the host to [d, tok] tile layout, shared across all
cores) plus its weight slice, computes Y_c = X @ W_c + bias_c, and the
host concatenates the per-core outputs along F. No device collectives.

Numerics: operands are rounded to bf16 on the host; matmuls accumulate
in fp32 PSUM; outputs are stored as bf16 (halving output HBM traffic)
and upcast to fp32 on the host. Measured 2.9e-3 relative error at full
scale vs the 2e-2 gate. bf16 streams through the PE at the same 1 column/cycle as fp32r,
but (a) the whole 2048-feature weight slice fits SBUF in one pass
(16 MB), so activations stream once instead of twice, and (b) LDWEIGHTS
gets the compiler-automatic fast-weight-load path (fp32 is excluded
from FWL) and is amortized over 4 N=512 matmuls per x-tile instead
of 2.

Per-core program: weight slice [4096, 2048] resident in SBUF as
[128p, 32ks, 2048f] bf16 (128 KB/partition); activations streamed as
[4096, 128]-token column tiles (stationary operand); each 128-token
tile accumulates 32 k-subtiles into 4 fp32 PSUM banks (one per 512-wide
feature chunk, 4 matmuls per LDWEIGHTS). Bias is added during the
PSUM->SBUF eviction on the vector engine (which also casts to bf16);
per-tile outputs leave in one 0.5 MB DMA.

Measured (interleaved repeat-loop slope): 220 ns per N=512 matmul on a
single core (~97% of the 213 ns warm 2.4 GHz streaming floor; walrus
emits one LDWEIGHTS per matmul but the PE's pull-ahead hides it), and
~267 ns/MM with all 8 cores running concurrently -> 2.18-2.23 ms/core,
which sits at the sustained all-core clock floor (~1.9-2.0 GHz under
full load; 1->2->4->8-core sweep: 218/223/263/266 ns per matmul). The
fp32r predecessor measured 2.99 ms on the same harness.
"""

import numpy as np

# Problem constants (hardcoded per the task contract).
B, S, D, F = 4, 2048, 4096, 16384
T = B * S  # 8192 tokens
P = 128
NCORES = 8

FD = 512  # matmul free dim (one fp32 PSUM bank)
FC = F // NCORES  # 2048 features per core
KS = D // P  # 32 k-subtiles
NTT = T // P  # 64 token tiles
NFC = FC // FD  # 4 feature chunks

_COMPILED = None


def _build(repeat=1):
    import concourse.bacc as bacc
    import concourse.mybir as mybir
    import concourse.tile as tile

    DT = mybir.dt.bfloat16
    nc = bacc.Bacc("TRN2", target_bir_lowering=False, debug=False)

    xt = nc.dram_tensor("xt", (P, NTT, KS, P), DT, kind="ExternalInput")
    w = nc.dram_tensor("w", (P, KS, FC), DT, kind="ExternalInput")
    bias = nc.dram_tensor("bias", (P, FC), mybir.dt.float32, kind="ExternalInput")
    y = nc.dram_tensor(
        "y", (P, NTT, NFC, FD), DT, kind="ExternalOutput"
    )

    with tile.TileContext(nc) as tc:
        with (
            tc.tile_pool(name="wpool", bufs=1) as wpool,
            tc.tile_pool(name="bpool", bufs=1) as bpool,
            tc.tile_pool(name="xpool", bufs=4) as xpool,
            tc.tile_pool(name="opool", bufs=2) as opool,
            tc.tile_pool(name="pspool", bufs=8, space="PSUM") as pspool,
        ):
            # W/bias are loaded once, outside the repeat loop: resident for
            # the kernel's lifetime (matches real single-shot use, and keeps
            # the repeat-timing loop free of an artificial W reload).
            b_sb = bpool.tile([P, FC], mybir.dt.float32, name="b_sb")
            nc.sync.dma_start(out=b_sb[:], in_=bias[:, :])
            w_sb = wpool.tile([P, KS, FC], DT, name="w_sb")
            gs = KS // 8
            for g in range(8):
                nc.sync.dma_start(
                    out=w_sb[:, g * gs : (g + 1) * gs, :],
                    in_=w[:, g * gs : (g + 1) * gs, :],
                )

            def body():
                for tt in range(NTT):
                    x_sb = xpool.tile([P, KS, P], DT, name="x_sb")
                    nc.sync.dma_start(out=x_sb[:], in_=xt[:, tt, :, :])
                    psums = [
                        pspool.tile([P, FD], mybir.dt.float32, name="ps")
                        for _ in range(NFC)
                    ]
                    for ks in range(KS):
                        for fc in range(NFC):
                            nc.tensor.matmul(
                                psums[fc][:],
                                lhsT=x_sb[:, ks, :],
                                rhs=w_sb[:, ks, fc * FD : (fc + 1) * FD],
                                start=(ks == 0),
                                stop=(ks == KS - 1),
                            )
                    o_sb = opool.tile([P, NFC, FD], DT, name="o_sb")
                    for fc in range(NFC):
                        nc.vector.tensor_tensor(
                            out=o_sb[:, fc, :],
                            in0=psums[fc][:],
                            in1=b_sb[:, fc * FD : (fc + 1) * FD],
                            op=mybir.AluOpType.add,
                        )
                    nc.sync.dma_start(out=y[:, tt, :, :], in_=o_sb[:])

            if repeat == 1:
                body()
            else:
                with tc.For_i(0, repeat, 1):
                    body()

    nc.compile()
    return nc


def _get_compiled():
    global _COMPILED
    if _COMPILED is None:
        _COMPILED = _build()
    return _COMPILED


def prep_inputs(inputs, kernel, bias):
    import ml_dtypes

    bf16 = ml_dtypes.bfloat16
    x32 = np.asarray(inputs, dtype=np.float32).reshape(T, D)
    # xt[p, tt, ks, t] = X[tt*128+t, ks*128+p]
    xt_host = np.ascontiguousarray(
        x32.reshape(NTT, P, KS, P).transpose(3, 0, 2, 1).astype(bf16)
    )
    w32 = np.asarray(kernel, dtype=np.float32)
    # w[p, ks, f] = W[ks*128+p, f]
    w_host = np.ascontiguousarray(
        w32.reshape(KS, P, F).transpose(1, 0, 2).astype(bf16)
    )
    b32 = np.asarray(bias, dtype=np.float32)
    in_maps = []
    for c in range(NCORES):
        in_maps.append(
            {
                "xt": xt_host,
                "w": np.ascontiguousarray(w_host[:, :, c * FC : (c + 1) * FC]),
                "bias": np.ascontiguousarray(
                    np.broadcast_to(b32[c * FC : (c + 1) * FC], (P, FC))
                ),
            }
        )
    return in_maps


def gather(results):
    out = np.empty((T, F), dtype=np.float32)
    for c in range(NCORES):
        y_c = results[c]["y"]  # [P, NTT, NFC, FD]
        out[:, c * FC : (c + 1) * FC] = (
            y_c.reshape(P, NTT, FC).transpose(1, 0, 2).reshape(T, FC)
        ).astype(np.float32)
    return out.reshape(B, S, F)


def kernel(**inputs):
    from concourse import bass_utils

    nc = _get_compiled()
    in_maps = prep_inputs(inputs["inputs"], inputs["kernel"], inputs["bias"])
    last_err = None
    for _attempt in range(3):
        try:
            res = bass_utils.run_bass_kernel_spmd(
                nc, in_maps, core_ids=list(range(NCORES)), trace=False
            )
            return gather(res.results)
        except Exception as e:  # transient NRT/axon errors observed ~rarely
            last_err = e
    raise last_err



# revision 2
# speedup vs baseline: 1.1274x; 1.1274x over previous
"""Dense MLP kernel for Trainium2: y = inputs @ kernel + bias.

Full shapes: inputs (4, 2048, 4096) f32, kernel (4096, 16384) f32,
bias (16384,) f32 -> y (4, 2048, 16384) f32.

Strategy: tensor-parallel over the output feature dim F=16384, split 8
ways (2048 features per core). Each core receives the full activations
(pre-transposed on the host to [d, tok] tile layout), computes
Y_c = X @ W_c + bias_c, and the host concatenates along F. No device
collectives.

Numerics / speed: mixed-precision contraction. Of the 32 k-subtiles
(128 each), NK=8 run as fp8-e4m3 DoubleRow matmuls (2 k-subtiles per
matmul at 1 column-pair/cycle -> 2x bf16 MAC rate; measured 266.8 ns
per K=256,N=512 matmul vs 264.6 ns for the bf16 K=128 equivalent, i.e.
the full 2x materializes) and the remaining 24 run in bf16. All
accumulate into the same fp32 PSUM bank. Error is dominated by the
e4m3 quantization of the DR fraction: rel_err = 3.76e-2 * sqrt(8/32)
~= 1.896e-2, measured on HW at 1.8957e-2 (CPU-predicted 1.8957e-2 -
deterministic for the fixed seed) vs the 2e-2 gate.

Scale management (zero device cost): PSUM holds 64*y. The bf16-part
activations are pre-scaled x64 on the host (exact bf16 exponent
shift); the e4m3 parts use X*8 and W*8 (8*8=64), which also lifts the
weights (sigma=1/64) out of the e4m3 subnormal range. Bias is
pre-scaled x64; outputs leave as bf16 (64*y, exact shift) and the host
multiplies by 1/64 during the fp32 upcast.

Per-core program: weight slices resident in SBUF (w8 [128,8,2048] e4m3
16KB/part + wb [128,24,2048] bf16 96KB/part); activations streamed as
128-token column tiles (stationary operand, double-buffered x4); each
tile accumulates 4 DR + 24 bf16 matmuls into 4 fp32 PSUM banks (one
per 512-wide feature chunk); bias added during PSUM->SBUF eviction on
the vector engine (also casts to bf16); per-tile outputs leave in one
0.5 MB DMA.

Measured (interleaved repeat-loop slope, all 8 cores concurrent):
1.896 ms/core = 112 MMs/tile * 64 tiles * ~264.6 ns/MM, at the
sustained all-core issue rate (~1.95 GHz effective; single-core runs
~2.38 GHz but >=4 busy cores throttle - operand-width experiments
showed the throttle is not datapath-power driven). The bf16
predecessor measured 2.168 ms on the same harness; pure-DR (all 32
k-subtiles fp8) measures 1.093 ms but fails the error gate at 3.76e-2.
"""

import numpy as np

# Problem constants (hardcoded per the task contract).
B, S, D, F = 4, 2048, 4096, 16384
T = B * S  # 8192 tokens
P = 128
NCORES = 8

FD = 512  # matmul free dim (one fp32 PSUM bank)
FC = F // NCORES  # 2048 features per core
KS = D // P  # 32 k-subtiles
NTT = T // P  # 64 token tiles
NFC = FC // FD  # 4 feature chunks

NK = 8  # k-subtiles computed in e4m3 DoubleRow (must be even)
KSB = KS - NK  # k-subtiles computed in bf16
SX = 8.0  # e4m3 activation pre-scale
SW = 8.0  # e4m3 weight pre-scale (SX*SW == 64)
OSCALE = 1.0 / 64.0  # host-side output unscale

_COMPILED = None


def _build(repeat=1):
    import concourse.bacc as bacc
    import concourse.mybir as mybir
    import concourse.tile as tile

    nc = bacc.Bacc("TRN2", target_bir_lowering=False, debug=False)

    xt8 = nc.dram_tensor(
        "xt8", (P, NTT, NK, P), mybir.dt.float8e4, kind="ExternalInput"
    )
    xtb = nc.dram_tensor(
        "xtb", (P, NTT, KSB, P), mybir.dt.bfloat16, kind="ExternalInput"
    )
    w8 = nc.dram_tensor("w8", (P, NK, FC), mybir.dt.float8e4, kind="ExternalInput")
    wb = nc.dram_tensor("wb", (P, KSB, FC), mybir.dt.bfloat16, kind="ExternalInput")
    bias = nc.dram_tensor("bias", (P, FC), mybir.dt.float32, kind="ExternalInput")
    y = nc.dram_tensor("y", (P, NTT, NFC, FD), mybir.dt.bfloat16, kind="ExternalOutput")

    DR = mybir.MatmulPerfMode.DoubleRow

    with tile.TileContext(nc) as tc:
        with (
            tc.tile_pool(name="wpool", bufs=1) as wpool,
            tc.tile_pool(name="bpool", bufs=1) as bpool,
            tc.tile_pool(name="xpool", bufs=4) as xpool,
            tc.tile_pool(name="x8pool", bufs=4) as x8pool,
            tc.tile_pool(name="opool", bufs=2) as opool,
            tc.tile_pool(name="pspool", bufs=8, space="PSUM") as pspool,
        ):
            # Weights/bias load once, outside the repeat loop: resident for
            # the kernel's lifetime.
            b_sb = bpool.tile([P, FC], mybir.dt.float32, name="b_sb")
            nc.sync.dma_start(out=b_sb[:], in_=bias[:, :])
            w8_sb = wpool.tile([P, NK, FC], mybir.dt.float8e4, name="w8_sb")
            for g in range(NK // 2):
                nc.sync.dma_start(
                    out=w8_sb[:, g * 2 : (g + 1) * 2, :],
                    in_=w8[:, g * 2 : (g + 1) * 2, :],
                )
            wb_sb = wpool.tile([P, KSB, FC], mybir.dt.bfloat16, name="wb_sb")
            for g in range(KSB):
                nc.sync.dma_start(out=wb_sb[:, g : g + 1, :], in_=wb[:, g : g + 1, :])

            def body():
                for tt in range(NTT):
                    x8_sb = x8pool.tile([P, NK, P], mybir.dt.float8e4, name="x8_sb")
                    nc.sync.dma_start(out=x8_sb[:], in_=xt8[:, tt, :, :])
                    xb_sb = xpool.tile([P, KSB, P], mybir.dt.bfloat16, name="xb_sb")
                    nc.sync.dma_start(out=xb_sb[:], in_=xtb[:, tt, :, :])
                    psums = [
                        pspool.tile([P, FD], mybir.dt.float32, name="ps")
                        for _ in range(NFC)
                    ]
                    for i in range(NK // 2):
                        for fc in range(NFC):
                            nc.tensor.matmul(
                                psums[fc][:],
                                lhsT=x8_sb[:, 2 * i : 2 * i + 2, :],
                                rhs=w8_sb[
                                    :, 2 * i : 2 * i + 2, fc * FD : (fc + 1) * FD
                                ],
                                start=(i == 0),
                                stop=False,
                                perf_mode=DR,
                            )
                    for ks in range(KSB):
                        for fc in range(NFC):
                            nc.tensor.matmul(
                                psums[fc][:],
                                lhsT=xb_sb[:, ks, :],
                                rhs=wb_sb[:, ks, fc * FD : (fc + 1) * FD],
                                start=False,
                                stop=(ks == KSB - 1),
                            )
                    o_sb = opool.tile([P, NFC, FD], mybir.dt.bfloat16, name="o_sb")
                    for fc in range(NFC):
                        nc.vector.tensor_tensor(
                            out=o_sb[:, fc, :],
                            in0=psums[fc][:],
                            in1=b_sb[:, fc * FD : (fc + 1) * FD],
                            op=mybir.AluOpType.add,
                        )
                    nc.sync.dma_start(out=y[:, tt, :, :], in_=o_sb[:])

            if repeat == 1:
                body()
            else:
                with tc.For_i(0, repeat, 1):
                    body()

    nc.compile()
    return nc


def _get_compiled():
    global _COMPILED
    if _COMPILED is None:
        _COMPILED = _build()
    return _COMPILED


def prep_inputs(inputs, kernel, bias):
    import ml_dtypes

    e4m3 = ml_dtypes.float8_e4m3
    bf16 = ml_dtypes.bfloat16

    x32 = np.asarray(inputs, dtype=np.float32).reshape(T, D)
    # [p, tt, ks, t] layout: xt[p, tt, ks, t] = X[tt*128+t, ks*128+p]
    xt_all = x32.reshape(NTT, P, KS, P).transpose(3, 0, 2, 1)
    xt8_host = np.ascontiguousarray((xt_all[:, :, :NK, :] * SX).astype(e4m3))
    xtb_host = np.ascontiguousarray((xt_all[:, :, NK:, :] * 64.0).astype(bf16))

    w32 = np.asarray(kernel, dtype=np.float32)
    w_all = w32.reshape(KS, P, F).transpose(1, 0, 2)  # w[p, ks, f] = W[ks*128+p, f]
    w8_host = np.ascontiguousarray((w_all[:, :NK, :] * SW).astype(e4m3))
    wb_host = np.ascontiguousarray(w_all[:, NK:, :].astype(bf16))

    b32 = np.asarray(bias, dtype=np.float32) * 64.0
    in_maps = []
    for c in range(NCORES):
        sl = slice(c * FC, (c + 1) * FC)
        in_maps.append(
            {
                "xt8": xt8_host,
                "xtb": xtb_host,
                "w8": np.ascontiguousarray(w8_host[:, :, sl]),
                "wb": np.ascontiguousarray(wb_host[:, :, sl]),
                "bias": np.ascontiguousarray(np.broadcast_to(b32[sl], (P, FC))),
            }
        )
    return in_maps


def gather(results):
    out = np.empty((T, F), dtype=np.float32)
    for c in range(NCORES):
        y_c = results[c]["y"]  # [P, NTT, NFC, FD], bf16, 64*y
        out[:, c * FC : (c + 1) * FC] = (
            y_c.reshape(P, NTT, FC).transpose(1, 0, 2).reshape(T, FC)
        ).astype(np.float32)
    out *= OSCALE
    return out.reshape(B, S, F)


def kernel(**inputs):
    from concourse import bass_utils

    nc = _get_compiled()
    in_maps = prep_inputs(inputs["inputs"], inputs["kernel"], inputs["bias"])
    last_err = None
    for _attempt in range(3):
        try:
            res = bass_utils.run_bass_kernel_spmd(
                nc, in_maps, core_ids=list(range(NCORES)), trace=False
            )
            return gather(res.results)
        except Exception as e:  # transient NRT/axon errors observed ~rarely
            last_err = e
    raise last_err
